# revision 1
# baseline (speedup 1.0000x reference)
"""AttentionResidualGRU fused Trainium2 kernel — feature-major, v2.

Data parallel over batch (8 cores x 32 rows). All state kept feature-major
(partition = hidden/gate dim, free = batch), so the recurrent matmuls are
weight-stationary: per step 12 gate M-tiles x 4 h K-tiles of
(LDWEIGHTS [128,128]bf16 + MATMUL N=32) which the PE sustains at ~27-35ns
per pair, and h_{t+1} comes out of the gate math already in rhs layout for
the next step — no PE transposes anywhere.

Input projections gx(+biases) are folded into the same PSUM accumulation as
12 extra K=3 matmuls per step against the [3, 32T] v-buffer (rows
[1, v0, v1]); only the first matmul of each bank carries start=True so
per-element has_written semantics make every region overwrite-then-
accumulate correctly.

PSUM banks (8): rA x2 = r pre-acts, zA x2 = z pre-acts, B x2 = gh_n + b_hn,
gN x1 = gx_n, HLE x1 = attention hidden | logit rows [ld,-ld] | E rows
[res0,res1|v0,v1]. Gate tail uses oz = sigmoid(-z_pre) = 1-z so
h' = (h - oz*h) + oz*n. Scan2's head runs in the prev-step PE tail and its elementwise
tail on DVE right behind the GRU chain;
x history lives in the bf16 output buffer and doubles as the attention rhs.
"""

import os
import sys

import numpy as np
import ml_dtypes

BF16 = ml_dtypes.bfloat16

for _p in ("/opt/trn_rl_repo", "/root/.axon_site/_ro/trn_rl_repo"):
    if os.path.isdir(_p) and _p not in sys.path:
        sys.path.append(_p)

B, T, H, IN, OUT = 256, 1024, 512, 2, 2
NCORES = 8
Bc = B // NCORES          # 32
SV_COLS = Bc * T          # 32768

_PROG_CACHE = {}


# ----------------------------------------------------------------- host prep

def _prep_consts(W_ih, W_hh, b_ih, b_hh, Wa1, ba1, Wa2, ba2, Wr, br):
    f = np.float32
    W_ih = np.asarray(W_ih, f); W_hh = np.asarray(W_hh, f)
    b_ih = np.asarray(b_ih, f); b_hh = np.asarray(b_hh, f)
    Wa1 = np.asarray(Wa1, f); ba1 = np.asarray(ba1, f)
    Wa2 = np.asarray(Wa2, f); ba2 = np.asarray(ba2, f)
    Wr = np.asarray(Wr, f); br = np.asarray(br, f)

    def gate_row0(m):
        return 128 * m if m < 4 else (512 + 128 * (m - 4) if m < 8
                                      else 1024 + 128 * (m - 8))

    # 48 stationary gate tiles: tile (m, k) at cols 128*(4m+k).
    wt = np.zeros((128, 48 * 128), f)
    for m in range(12):
        r0 = gate_row0(m)
        for k in range(4):
            wt[:, 128 * (4 * m + k):128 * (4 * m + k + 1)] = \
                W_hh[r0:r0 + 128, 128 * k:128 * (k + 1)].T

    b4 = np.zeros((4, 128), f)       # b_hh n-part, chunk k in row k
    for k in range(4):
        b4[k] = b_hh[1024 + 128 * k:1024 + 128 * (k + 1)]
    sel4 = np.zeros((4, 128), f)     # chunk selector rhs
    for c in range(4):
        sel4[c, 32 * c:32 * (c + 1)] = 1.0

    # in-loop gx stationaries: [3, 128] per chunk c (rows [bias, W0, W1])
    wx = np.zeros((3, 12 * 128), f)
    for c in range(12):
        r0 = gate_row0(c)
        bias = b_ih[r0:r0 + 128] + (b_hh[r0:r0 + 128] if c < 8 else 0.0)
        wx[0, 128 * c:128 * (c + 1)] = bias
        wx[1, 128 * c:128 * (c + 1)] = W_ih[r0:r0 + 128, 0]
        wx[2, 128 * c:128 * (c + 1)] = W_ih[r0:r0 + 128, 1]

    # E matmuls: E2 = [res0, res1 | v0, v1]
    ew = np.zeros((128, 8), f)       # k-tile k at cols 2k: [Wr0, Wr1]
    for k in range(4):
        ew[:, 2 * k + 0] = Wr[0, 128 * k:128 * (k + 1)]
        ew[:, 2 * k + 1] = Wr[1, 128 * k:128 * (k + 1)]
    eaP = np.zeros((3, 2), f)        # over [1, v0, v1]: br row
    eaP[0] = [br[0], br[1]]
    eaQ = np.zeros((3, 2), f)        # [v0, v1]
    eaQ[1, 0] = 1.0
    eaQ[2, 1] = 1.0

    # attention MLP split: v-part [3, 128]x2 (rows [ba1, Wa1v0, Wa1v1]),
    # x-part [2, 128]x2 (rows [Wa1x0, Wa1x1])
    wa1v = np.zeros((3, 256), f)
    wa1x = np.zeros((2, 256), f)
    for j in range(2):
        sl = slice(128 * j, 128 * (j + 1))
        wa1v[0, sl] = ba1[sl]
        wa1v[1, sl] = Wa1[sl, 2]
        wa1v[2, sl] = Wa1[sl, 3]
        wa1x[0, sl] = Wa1[sl, 0]
        wa1x[1, sl] = Wa1[sl, 1]
    wd = (Wa2[0] - Wa2[1]).astype(f)  # [256]
    wd2 = np.zeros((128, 4), f)       # k-tile k at cols 2k: [wd, -wd]
    for k in range(2):
        wd2[:, 2 * k + 0] = wd[128 * k:128 * (k + 1)]
        wd2[:, 2 * k + 1] = -wd[128 * k:128 * (k + 1)]
    db = float(ba2[0] - ba2[1])
    dbias = np.array([[db], [-db]], f)

    return dict(wt=wt.astype(BF16),
                b4=b4.astype(BF16), sel4=sel4.astype(BF16),
                wx=wx.astype(BF16), ew=ew.astype(BF16),
                eaP=eaP.astype(BF16), eaQ=eaQ.astype(BF16),
                wa1v=wa1v.astype(BF16), wa1x=wa1x.astype(BF16),
                wd2=wd2.astype(BF16), dbias=dbias)


def _prep_core(c, X0, V):
    f = np.float32
    bs = slice(Bc * c, Bc * (c + 1))
    Vc = np.asarray(V[bs], f)                      # [32, T, 2]
    sv3 = np.zeros((3, SV_COLS), f)                # rows [1, v0, v1], col 32t+b
    sv3[0] = 1.0
    sv3[1] = Vc[:, :, 0].T.reshape(-1)
    sv3[2] = Vc[:, :, 1].T.reshape(-1)
    xf0 = np.asarray(X0[bs], f).T.copy()           # [2, 32] f32
    return dict(sv3=sv3.astype(BF16), xf0=xf0, xb0=xf0.astype(BF16))


# ------------------------------------------------------------- device program

def _build_program():
    from concourse import bacc, tile, mybir  # noqa

    f32 = mybir.dt.float32
    bf16 = mybir.dt.bfloat16
    AF = mybir.ActivationFunctionType

    nc = bacc.Bacc(None)
    d_wt = nc.declare_dram_parameter("wt", [128, 48 * 128], bf16, isOutput=False)
    d_b4 = nc.declare_dram_parameter("b4", [4, 128], bf16, isOutput=False)
    d_sel = nc.declare_dram_parameter("sel4", [4, 128], bf16, isOutput=False)
    d_wx = nc.declare_dram_parameter("wx", [3, 12 * 128], bf16, isOutput=False)
    d_ew = nc.declare_dram_parameter("ew", [128, 8], bf16, isOutput=False)
    d_eaP = nc.declare_dram_parameter("eaP", [3, 2], bf16, isOutput=False)
    d_eaQ = nc.declare_dram_parameter("eaQ", [3, 2], bf16, isOutput=False)
    d_wa1v = nc.declare_dram_parameter("wa1v", [3, 256], bf16, isOutput=False)
    d_wa1x = nc.declare_dram_parameter("wa1x", [2, 256], bf16, isOutput=False)
    d_wd2 = nc.declare_dram_parameter("wd2", [128, 4], bf16, isOutput=False)
    d_db = nc.declare_dram_parameter("dbias", [2, 1], f32, isOutput=False)
    d_sv = nc.declare_dram_parameter("sv3", [3, SV_COLS], bf16, isOutput=False)
    d_xf0 = nc.declare_dram_parameter("xf0", [2, Bc], f32, isOutput=False)
    d_xb0 = nc.declare_dram_parameter("xb0", [2, Bc], bf16, isOutput=False)
    d_out = nc.declare_dram_parameter("out", [2, T * Bc], bf16, isOutput=True)

    with tile.TileContext(nc) as tc:
        with (
            tc.tile_pool(name="const", bufs=1) as cpool,
            tc.tile_pool(name="state", bufs=1) as spool,
            tc.tile_pool(name="hpool", bufs=2) as hpool,
            tc.tile_pool(name="work", bufs=2) as wpool,
            tc.tile_pool(name="pR", bufs=2, space="PSUM") as pR,
            tc.tile_pool(name="pZ", bufs=2, space="PSUM") as pZ,
            tc.tile_pool(name="pB", bufs=2, space="PSUM") as pB,
            tc.tile_pool(name="pG", bufs=1, space="PSUM") as pG,
            tc.tile_pool(name="pHL", bufs=1, space="PSUM") as pHL,
        ):
            # ---- constants
            wt = cpool.tile([128, 48 * 128], bf16, tag="wt")
            b4_t = cpool.tile([4, 128], bf16, tag="b4")
            sel_t = cpool.tile([4, 128], bf16, tag="sel4")
            wx_t = cpool.tile([3, 12 * 128], bf16, tag="wx")
            ew_t = cpool.tile([128, 8], bf16, tag="ew")
            eaP_t = cpool.tile([3, 2], bf16, tag="eaP")
            eaQ_t = cpool.tile([3, 2], bf16, tag="eaQ")
            wa1v_t = cpool.tile([3, 256], bf16, tag="wa1v")
            wa1x_t = cpool.tile([2, 256], bf16, tag="wa1x")
            wd2_t = cpool.tile([128, 4], bf16, tag="wd2")
            db_t = cpool.tile([2, 1], f32, tag="dbias")
            sv_t = cpool.tile([3, SV_COLS], bf16, tag="sv3")
            for dst, src in ((wt, d_wt), (b4_t, d_b4),
                             (sel_t, d_sel), (wx_t, d_wx), (ew_t, d_ew),
                             (eaP_t, d_eaP), (eaQ_t, d_eaQ), (wa1v_t, d_wa1v),
                             (wa1x_t, d_wa1x), (wd2_t, d_wd2), (db_t, d_db),
                             (sv_t, d_sv)):
                nc.sync.dma_start(out=dst[:], in_=src[:])

            # ---- state
            xf = spool.tile([2, Bc], f32, tag="xf")
            outb = spool.tile([2, T * Bc], bf16, tag="outb")
            xb0 = spool.tile([2, Bc], bf16, tag="xb0")
            nc.sync.dma_start(out=xf[:], in_=d_xf0[:])
            nc.sync.dma_start(out=xb0[:], in_=d_xb0[:])

            mm = nc.tensor.matmul

            h = hpool.tile([128, 128], bf16, tag="h")
            nc.vector.memset(h[:], 0.0)

            for s in range(T + 1):
                last = s == T
                svs = sv_t[:, Bc * s:Bc * (s + 1)] if not last else None
                svp = sv_t[:, Bc * (s - 1):Bc * s]  # v_{s-1} (s>=1)
                h_prev = h

                # 1) h-independent PE work in the prev-step tail
                if not last:
                    rA = pR.tile([128, 128], f32, tag="rA")
                    zA = pZ.tile([128, 128], f32, tag="zA")
                    gN = pG.tile([128, 128], f32, tag="gN")
                    bankB = pB.tile([128, 128], f32, tag="B")
                    for m in range(12):
                        dstA = (rA[:, 32 * m:32 * (m + 1)] if m < 4 else
                                zA[:, 32 * (m - 4):32 * (m - 3)] if m < 8 else
                                gN[:, 32 * (m - 8):32 * (m - 7)])
                        mm(dstA, wx_t[:, 128 * m:128 * (m + 1)], svs,
                           start=(m % 4 == 0), stop=(m >= 8))
                    mm(bankB[:], b4_t[:], sel_t[:], start=True, stop=False)
                if s >= 1:
                    hl = pHL.tile([128, 160], f32, tag="HL")
                    for j in range(2):
                        mm(hl[:, 32 * j:32 * (j + 1)],
                           wa1v_t[:, 128 * j:128 * (j + 1)], svp,
                           start=True, stop=False)
                    xprev = (xb0[:] if s == 1
                             else outb[:, Bc * (s - 2):Bc * (s - 1)])
                    for j in range(2):
                        mm(hl[:, 32 * j:32 * (j + 1)],
                           wa1x_t[:, 128 * j:128 * (j + 1)], xprev,
                           start=False, stop=True)
                    hid = wpool.tile([128, 64], bf16, tag="hid")
                    nc.scalar.activation(hid[:], hl[:, 0:64], AF.Relu)
                    for k in range(2):
                        mm(hl[0:2, 64:96], wd2_t[:, 2 * k:2 * (k + 1)],
                           hid[:, 32 * k:32 * (k + 1)],
                           start=(k == 0), stop=(k == 1))
                    aw = wpool.tile([2, Bc], f32, tag="aw")
                    nc.scalar.activation(aw[:], hl[0:2, 64:96], AF.Sigmoid,
                                         bias=db_t[:])

                # 2) n-part and r-part recurrent pairs + gate head
                if not last:
                    for m in (8, 9, 10, 11, 0, 1, 2, 3):
                        dst = (rA[:, 32 * m:32 * (m + 1)] if m < 4
                               else bankB[:, 32 * (m - 8):32 * (m - 7)])
                        for k in range(4):
                            mm(dst,
                               wt[:, 128 * (4 * m + k):128 * (4 * m + k + 1)],
                               h[:, 32 * k:32 * (k + 1)],
                               start=False, stop=(k == 3))
                    rz = wpool.tile([128, 256], bf16, tag="rz")
                    nc.scalar.activation(rz[:, 0:128], rA[:], AF.Sigmoid)
                    u = wpool.tile([128, 128], f32, tag="u")
                    nc.vector.tensor_mul(u[:], rz[:, 0:128], bankB[:])
                    nc.vector.tensor_add(u[:], u[:], gN[:])

                # 4) E rows from h_s + stage to SBUF
                if s >= 1:
                    for k in range(4):
                        mm(hl[0:2, 96:96 + Bc], ew_t[:, 2 * k:2 * (k + 1)],
                           h_prev[:, 32 * k:32 * (k + 1)],
                           start=(k == 0), stop=False)
                    mm(hl[0:2, 96:96 + Bc], eaP_t[:], svp,
                       start=False, stop=True)
                    mm(hl[0:2, 96 + Bc:96 + 2 * Bc], eaQ_t[:], svp,
                       start=True, stop=True)
                    e2s = wpool.tile([2, 2 * Bc], f32, tag="e2s")
                    nc.scalar.copy(e2s[:], hl[0:2, 96:96 + 2 * Bc])

                # 5) z-part pairs + gate tail
                if not last:
                    for m in (4, 5, 6, 7):
                        for k in range(4):
                            mm(zA[:, 32 * (m - 4):32 * (m - 3)],
                               wt[:, 128 * (4 * m + k):128 * (4 * m + k + 1)],
                               h[:, 32 * k:32 * (k + 1)],
                               start=False, stop=(k == 3))
                    nc.scalar.activation(rz[:, 128:256], zA[:],
                                         AF.Sigmoid, scale=-1.0)
                    n_sb = wpool.tile([128, 128], bf16, tag="n_sb")
                    nc.scalar.activation(n_sb[:], u[:], AF.Tanh)
                    q = wpool.tile([128, 128], bf16, tag="q")
                    nc.vector.tensor_mul(q[:], rz[:, 128:256], h[:])
                    nc.vector.tensor_sub(q[:], h[:], q[:])
                    nc.vector.tensor_mul(n_sb[:], rz[:, 128:256], n_sb[:])
                    h = hpool.tile([128, 128], bf16, tag="h")
                    nc.vector.tensor_add(h[:], q[:], n_sb[:])

                # 6) scan2 tail on GpSimd
                if s >= 1:
                    t01 = wpool.tile([2, Bc], f32, tag="t01")
                    nc.vector.tensor_mul(t01[:], aw[:], e2s[:, 0:Bc])
                    nc.vector.tensor_add(t01[:], t01[:], e2s[:, Bc:2 * Bc])
                    nc.vector.tensor_add(xf[:], xf[:], t01[:])
                    nc.vector.tensor_copy(outb[:, Bc * (s - 1):Bc * s], xf[:])

            nc.sync.dma_start(out=d_out[:], in_=outb[:])

    nc.compile()
    return nc


# ------------------------------------------------------------------ interface

def kernel(X0, V, W_ih, W_hh, b_ih, b_hh, Wa1, ba1, Wa2, ba2, Wr, br,
           _trace=False, _tmpdir=None):
    from concourse.bass_utils import run_bass_kernel_spmd

    if "prog" not in _PROG_CACHE:
        _PROG_CACHE["prog"] = _build_program()
    nc = _PROG_CACHE["prog"]

    consts = _prep_consts(W_ih, W_hh, b_ih, b_hh, Wa1, ba1, Wa2, ba2, Wr, br)
    in_maps = []
    for c in range(NCORES):
        core = _prep_core(c, X0, V)
        in_maps.append({**consts, **core})

    res = run_bass_kernel_spmd(nc, in_maps, list(range(NCORES)),
                               trace=_trace, tmpdir=_tmpdir)
    outs = []
    for c in range(NCORES):
        buf = np.asarray(res.results[c]["out"], dtype=np.float32)  # [2, T*Bc]
        outs.append(buf.reshape(2, T, Bc).transpose(2, 1, 0))
    out = np.concatenate(outs, axis=0)
    if _trace:
        return out, res
    return out



# revision 3
# speedup vs baseline: 1.0846x; 1.0846x over previous
"""AttentionResidualGRU fused Trainium2 kernel — v7: latency-ordered.

Data parallel over batch (8 cores x 32 rows), feature-major state
(partition = feature, free = batch), weight-stationary recurrent matmuls
(48 LDW+MM pairs/step at the ~27ns N<=64 dispatch floor).

The per-step schedule is built around the loop-carried h chain, with the
ASAP tile scheduler (dep-driven semaphores instead of sim-placement):
  - PE pair order r(16), n(16), z(16): sigmoid(r) fires after only 16
    pairs, and the B bank (b_hn + gh_n) lands exactly when u = r*B + gN
    needs it. r-pairs are k-grouped in halves so the first 8 need only
    h chunks 0-1, which the split gate tail produces early.
  - rA, zA, [gN|B], hl each get their own full PSUM bank (bufs=2):
    PSUM dependency tracking is tile-granular, so sharing a tile would
    chain sigmoid(r) behind the z pairs.
  - sigma(-z) yields oz = 1-z directly; the post-tanh tail is
    t2 = oz*n ; h' = t2 - nq (column halves), with nq = (oz-1)*h = -z*h
    built off-chain by one fused scalar_tensor_tensor while tanh runs.
  - relu runs on the DVE (tensor_scalar_max) in its idle window so the
    logits matmul never blocks the PE stream behind the Act FIFO, and
    scan2's elementwise tail runs on the DVE after h' (GpSimd's ~325ns
    per-op chain latency made it the critical x-loop).
  - gx input projections for step s+1 and the attention v-part ride in
    the PE idle window at the end of step s (PSUM bufs=2 rotation).
"""

import os
import sys

os.environ.setdefault("TILE_SCHEDULER", "asap")

import numpy as np
import ml_dtypes

BF16 = ml_dtypes.bfloat16

for _p in ("/opt/trn_rl_repo", "/root/.axon_site/_ro/trn_rl_repo"):
    if os.path.isdir(_p) and _p not in sys.path:
        sys.path.append(_p)

B, T, H, IN, OUT = 256, 1024, 512, 2, 2
NCORES = 8
Bc = B // NCORES          # 32
SV_COLS = Bc * T          # 32768

_PROG_CACHE = {}


# ----------------------------------------------------------------- host prep

def _prep_consts(W_ih, W_hh, b_ih, b_hh, Wa1, ba1, Wa2, ba2, Wr, br):
    f = np.float32
    W_ih = np.asarray(W_ih, f); W_hh = np.asarray(W_hh, f)
    b_ih = np.asarray(b_ih, f); b_hh = np.asarray(b_hh, f)
    Wa1 = np.asarray(Wa1, f); ba1 = np.asarray(ba1, f)
    Wa2 = np.asarray(Wa2, f); ba2 = np.asarray(ba2, f)
    Wr = np.asarray(Wr, f); br = np.asarray(br, f)

    def gate_row0(m):
        return 128 * m if m < 4 else (512 + 128 * (m - 4) if m < 8
                                      else 1024 + 128 * (m - 8))

    # 48 stationary gate tiles: tile (m, k) at cols 128*(4m+k).
    wt = np.zeros((128, 48 * 128), f)
    for m in range(12):
        r0 = gate_row0(m)
        for k in range(4):
            wt[:, 128 * (4 * m + k):128 * (4 * m + k + 1)] = \
                W_hh[r0:r0 + 128, 128 * k:128 * (k + 1)].T

    b4 = np.zeros((4, 128), f)       # b_hh n-part, chunk k in row k
    for k in range(4):
        b4[k] = b_hh[1024 + 128 * k:1024 + 128 * (k + 1)]
    sel4 = np.zeros((4, 128), f)     # chunk selector rhs
    for c in range(4):
        sel4[c, 32 * c:32 * (c + 1)] = 1.0

    # in-loop gx stationaries: [3, 128] per chunk c (rows [bias, W0, W1])
    wx = np.zeros((3, 12 * 128), f)
    for c in range(12):
        r0 = gate_row0(c)
        bias = b_ih[r0:r0 + 128] + (b_hh[r0:r0 + 128] if c < 8 else 0.0)
        wx[0, 128 * c:128 * (c + 1)] = bias
        wx[1, 128 * c:128 * (c + 1)] = W_ih[r0:r0 + 128, 0]
        wx[2, 128 * c:128 * (c + 1)] = W_ih[r0:r0 + 128, 1]

    # E matmuls: E2 = [res0, res1 | v0, v1]
    ew = np.zeros((128, 8), f)       # k-tile k at cols 2k: [Wr0, Wr1]
    for k in range(4):
        ew[:, 2 * k + 0] = Wr[0, 128 * k:128 * (k + 1)]
        ew[:, 2 * k + 1] = Wr[1, 128 * k:128 * (k + 1)]
    eaP = np.zeros((3, 2), f)        # over [1, v0, v1]: br row
    eaP[0] = [br[0], br[1]]
    eaQ = np.zeros((3, 2), f)        # [v0, v1]
    eaQ[1, 0] = 1.0
    eaQ[2, 1] = 1.0

    # attention MLP split: v-part [3, 128]x2 (rows [ba1, Wa1v0, Wa1v1]),
    # x-part [2, 128]x2 (rows [Wa1x0, Wa1x1])
    wa1v = np.zeros((3, 256), f)
    wa1x = np.zeros((2, 256), f)
    for j in range(2):
        sl = slice(128 * j, 128 * (j + 1))
        wa1v[0, sl] = ba1[sl]
        wa1v[1, sl] = Wa1[sl, 2]
        wa1v[2, sl] = Wa1[sl, 3]
        wa1x[0, sl] = Wa1[sl, 0]
        wa1x[1, sl] = Wa1[sl, 1]
    wd = (Wa2[0] - Wa2[1]).astype(f)  # [256]
    wd2 = np.zeros((128, 4), f)       # k-tile k at cols 2k: [wd, -wd]
    for k in range(2):
        wd2[:, 2 * k + 0] = wd[128 * k:128 * (k + 1)]
        wd2[:, 2 * k + 1] = -wd[128 * k:128 * (k + 1)]
    db = float(ba2[0] - ba2[1])
    dbias = np.array([[db], [-db]], f)

    return dict(wt=wt.astype(BF16),
                b4=b4.astype(BF16), sel4=sel4.astype(BF16),
                wx=wx.astype(BF16), ew=ew.astype(BF16),
                eaP=eaP.astype(BF16), eaQ=eaQ.astype(BF16),
                wa1v=wa1v.astype(BF16), wa1x=wa1x.astype(BF16),
                wd2=wd2.astype(BF16), dbias=dbias)


def _prep_core(c, X0, V):
    f = np.float32
    bs = slice(Bc * c, Bc * (c + 1))
    Vc = np.asarray(V[bs], f)                      # [32, T, 2]
    sv3 = np.zeros((3, SV_COLS), f)                # rows [1, v0, v1], col 32t+b
    sv3[0] = 1.0
    sv3[1] = Vc[:, :, 0].T.reshape(-1)
    sv3[2] = Vc[:, :, 1].T.reshape(-1)
    xf0 = np.asarray(X0[bs], f).T.copy()           # [2, 32] f32
    return dict(sv3=sv3.astype(BF16), xf0=xf0, xb0=xf0.astype(BF16))


# ------------------------------------------------------------- device program

def _build_program():
    from concourse import bacc, tile, mybir  # noqa

    f32 = mybir.dt.float32
    bf16 = mybir.dt.bfloat16
    AF = mybir.ActivationFunctionType
    ALU = mybir.AluOpType

    nc = bacc.Bacc(None)
    d_wt = nc.declare_dram_parameter("wt", [128, 48 * 128], bf16, isOutput=False)
    d_b4 = nc.declare_dram_parameter("b4", [4, 128], bf16, isOutput=False)
    d_sel = nc.declare_dram_parameter("sel4", [4, 128], bf16, isOutput=False)
    d_wx = nc.declare_dram_parameter("wx", [3, 12 * 128], bf16, isOutput=False)
    d_ew = nc.declare_dram_parameter("ew", [128, 8], bf16, isOutput=False)
    d_eaP = nc.declare_dram_parameter("eaP", [3, 2], bf16, isOutput=False)
    d_eaQ = nc.declare_dram_parameter("eaQ", [3, 2], bf16, isOutput=False)
    d_wa1v = nc.declare_dram_parameter("wa1v", [3, 256], bf16, isOutput=False)
    d_wa1x = nc.declare_dram_parameter("wa1x", [2, 256], bf16, isOutput=False)
    d_wd2 = nc.declare_dram_parameter("wd2", [128, 4], bf16, isOutput=False)
    d_db = nc.declare_dram_parameter("dbias", [2, 1], f32, isOutput=False)
    d_sv = nc.declare_dram_parameter("sv3", [3, SV_COLS], bf16, isOutput=False)
    d_xf0 = nc.declare_dram_parameter("xf0", [2, Bc], f32, isOutput=False)
    d_xb0 = nc.declare_dram_parameter("xb0", [2, Bc], bf16, isOutput=False)
    d_out = nc.declare_dram_parameter("out", [2, T * Bc], bf16, isOutput=True)

    with tile.TileContext(nc) as tc:
        with (
            tc.tile_pool(name="const", bufs=1) as cpool,
            tc.tile_pool(name="state", bufs=1) as spool,
            tc.tile_pool(name="hpool", bufs=2) as hpool,
            tc.tile_pool(name="work", bufs=2) as wpool,
            tc.tile_pool(name="pRA", bufs=2, space="PSUM") as pRA,
            tc.tile_pool(name="pZA", bufs=2, space="PSUM") as pZA,
            tc.tile_pool(name="pNB", bufs=2, space="PSUM") as pNB,
            tc.tile_pool(name="pHL", bufs=2, space="PSUM") as pHL,
        ):
            # ---- constants
            wt = cpool.tile([128, 48 * 128], bf16, tag="wt")
            b4_t = cpool.tile([4, 128], bf16, tag="b4")
            sel_t = cpool.tile([4, 128], bf16, tag="sel4")
            wx_t = cpool.tile([3, 12 * 128], bf16, tag="wx")
            ew_t = cpool.tile([128, 8], bf16, tag="ew")
            eaP_t = cpool.tile([3, 2], bf16, tag="eaP")
            eaQ_t = cpool.tile([3, 2], bf16, tag="eaQ")
            wa1v_t = cpool.tile([3, 256], bf16, tag="wa1v")
            wa1x_t = cpool.tile([2, 256], bf16, tag="wa1x")
            wd2_t = cpool.tile([128, 4], bf16, tag="wd2")
            db_t = cpool.tile([2, 1], f32, tag="dbias")
            sv_t = cpool.tile([3, SV_COLS], bf16, tag="sv3")
            for dst, src in ((wt, d_wt), (b4_t, d_b4),
                             (sel_t, d_sel), (wx_t, d_wx), (ew_t, d_ew),
                             (eaP_t, d_eaP), (eaQ_t, d_eaQ), (wa1v_t, d_wa1v),
                             (wa1x_t, d_wa1x), (wd2_t, d_wd2), (db_t, d_db),
                             (sv_t, d_sv)):
                nc.sync.dma_start(out=dst[:], in_=src[:])

            # ---- state
            xf = spool.tile([2, Bc], f32, tag="xf")
            outb = spool.tile([2, T * Bc], bf16, tag="outb")
            xb0 = spool.tile([2, Bc], bf16, tag="xb0")
            nc.sync.dma_start(out=xf[:], in_=d_xf0[:])
            nc.sync.dma_start(out=xb0[:], in_=d_xb0[:])

            mm = nc.tensor.matmul

            h = hpool.tile([128, 128], bf16, tag="h")
            nc.vector.memset(h[:], 0.0)

            # gate m-chunk layout in the shared PSUM banks (tiles padded to a
            # full 2KB bank so double-buffered tiles never share a bank —
            # start=True clears has_written bank-wide):
            #   rz[:, 0:128]   = rA (m 0..3)     rz[:, 128:256] = zA (m 4..7)
            #   nb[:, 0:128]   = gN (m 8..11)    nb[:, 128:256] = B  (gh_n+b_hn)
            def gx_emit(ra, za, nb, col):
                svs = sv_t[:, Bc * col:Bc * (col + 1)]
                for m in range(12):
                    if m < 4:
                        dst = ra[:, 32 * m:32 * (m + 1)]
                    elif m < 8:
                        dst = za[:, 32 * (m - 4):32 * (m - 3)]
                    else:
                        dst = nb[:, 32 * (m - 8):32 * (m - 7)]
                    mm(dst, wx_t[:, 128 * m:128 * (m + 1)], svs,
                       start=(m in (0, 4, 8)), stop=False)
                mm(nb[:, 128:256], b4_t[:], sel_t[:], start=False, stop=False)

            # preamble: gx for step 0
            ra_cur = pRA.tile([128, 512], f32, tag="ra")
            za_cur = pZA.tile([128, 512], f32, tag="za")
            nb_cur = pNB.tile([128, 512], f32, tag="nb")
            gx_emit(ra_cur, za_cur, nb_cur, 0)

            hl_cur = None
            for s in range(T + 1):
                gru = s < T

                # ---- PE: r(16), n(16), z(16) pairs ----
                if gru:
                    # r-pairs split by h-chunk halves: the first 8 read only
                    # h chunks 0-1 so they can start on the early h' half
                    for ks in ((0, 1), (2, 3)):
                        for m in (0, 1, 2, 3):
                            dst = ra_cur[:, 32 * m:32 * (m + 1)]
                            for k in ks:
                                mm(dst,
                                   wt[:, 128 * (4 * m + k):128 * (4 * m + k + 1)],
                                   h[:, 32 * k:32 * (k + 1)],
                                   start=False, stop=(k == 3))
                    for m in (8, 9, 10, 11, 4, 5, 6, 7):
                        if m >= 8:
                            dst = nb_cur[:, 128 + 32 * (m - 8):128 + 32 * (m - 7)]
                        else:
                            dst = za_cur[:, 32 * (m - 4):32 * (m - 3)]
                        for k in range(4):
                            mm(dst,
                               wt[:, 128 * (4 * m + k):128 * (4 * m + k + 1)],
                               h[:, 32 * k:32 * (k + 1)],
                               start=False, stop=(k == 3))

                # ---- PE: attention x-part, E rows (scan2 output s-1) ----
                if s >= 1:
                    svp = sv_t[:, Bc * (s - 1):Bc * s]
                    # x-part accumulates onto last iteration's v-part; must
                    # precede E's start=True (bank-wide has_written clear)
                    xprev = (xb0[:] if s == 1
                             else outb[:, Bc * (s - 2):Bc * (s - 1)])
                    for j in range(2):
                        mm(hl_cur[:, 32 * j:32 * (j + 1)],
                           wa1x_t[:, 128 * j:128 * (j + 1)], xprev,
                           start=False, stop=True)
                    # E rows from h_prev (= h before this step's update)
                    for k in range(4):
                        mm(hl_cur[0:2, 96:96 + Bc], ew_t[:, 2 * k:2 * (k + 1)],
                           h[:, 32 * k:32 * (k + 1)],
                           start=(k == 0), stop=False)
                    mm(hl_cur[0:2, 96:96 + Bc], eaP_t[:], svp,
                       start=False, stop=True)
                    mm(hl_cur[0:2, 96 + Bc:96 + 2 * Bc], eaQ_t[:], svp,
                       start=True, stop=True)

                # ---- PE: gx + B init for step s+1 ----
                if s + 1 < T:
                    ra_next = pRA.tile([128, 512], f32, tag="ra")
                    za_next = pZA.tile([128, 512], f32, tag="za")
                    nb_next = pNB.tile([128, 512], f32, tag="nb")
                    gx_emit(ra_next, za_next, nb_next, s + 1)
                else:
                    ra_next = za_next = nb_next = None

                # ---- PE: attention v-part for iteration s+1 (svp' = v_s) ----
                if s < T:
                    hl_next = pHL.tile([128, 512], f32, tag="HL")
                    svn = sv_t[:, Bc * s:Bc * (s + 1)]
                    for j in range(2):
                        mm(hl_next[:, 32 * j:32 * (j + 1)],
                           wa1v_t[:, 128 * j:128 * (j + 1)], svn,
                           start=True, stop=False)
                else:
                    hl_next = None

                # ---- gate head: sigmoids, u = r*B + gN ----
                if gru:
                    r_sb = wpool.tile([128, 128], bf16, tag="r_sb")
                    nc.scalar.activation(r_sb[:], ra_cur[:, 0:128], AF.Sigmoid)
                    oz = wpool.tile([128, 128], bf16, tag="oz")
                    nc.scalar.activation(oz[:], za_cur[:, 0:128],
                                         AF.Sigmoid, scale=-1.0)
                    t1 = wpool.tile([128, 128], f32, tag="t1")
                    nc.vector.tensor_mul(t1[:], r_sb[:], nb_cur[:, 128:256])
                    u = wpool.tile([128, 128], f32, tag="u")
                    nc.vector.tensor_add(u[:], t1[:], nb_cur[:, 0:128])

                # relu on DVE (fills its idle window; keeps Act FIFO short
                # and lets wd2 clear the PE stream early)
                if s >= 1:
                    hid = wpool.tile([128, 64], bf16, tag="hid")
                    nc.vector.tensor_scalar_max(hid[:], hl_cur[:, 0:64], 0.0)
                    for k in range(2):
                        mm(hl_cur[0:2, 64:96], wd2_t[:, 2 * k:2 * (k + 1)],
                           hid[:, 32 * k:32 * (k + 1)],
                           start=(k == 0), stop=(k == 1))

                if gru:
                    n_sb = wpool.tile([128, 128], bf16, tag="n_sb")
                    nc.scalar.activation(n_sb[:], u[:], AF.Tanh)
                    # off-chain while tanh runs: nq = (oz-1)*h = -z*h
                    nq = wpool.tile([128, 128], bf16, tag="nq")
                    nc.vector.scalar_tensor_tensor(
                        nq[:], oz[:], 1.0, h[:], ALU.subtract, ALU.mult)
                if s >= 1:
                    aw = wpool.tile([2, Bc], f32, tag="aw")
                    nc.scalar.activation(aw[:], hl_cur[0:2, 64:96], AF.Sigmoid,
                                         bias=db_t[:])

                # ---- gate tail: h' = oz*n - nq, column halves so the
                # first half of h' unblocks next iteration's k0/k1 pairs ----
                if gru:
                    t2 = wpool.tile([128, 128], bf16, tag="t2")
                    h_new = hpool.tile([128, 128], bf16, tag="h")
                    nc.vector.tensor_mul(t2[:, 0:64], oz[:, 0:64], n_sb[:, 0:64])
                    nc.vector.tensor_sub(h_new[:, 0:64], t2[:, 0:64],
                                         nq[:, 0:64])
                    nc.vector.tensor_mul(t2[:, 64:128], oz[:, 64:128],
                                         n_sb[:, 64:128])
                    nc.vector.tensor_sub(h_new[:, 64:128], t2[:, 64:128],
                                         nq[:, 64:128])

                # ---- scan2 elementwise tail (DVE idle window after h') ----
                if s >= 1:
                    p1 = wpool.tile([2, Bc], f32, tag="p1")
                    nc.vector.tensor_mul(p1[:], aw[:], hl_cur[0:2, 96:96 + Bc])
                    s2 = wpool.tile([2, Bc], f32, tag="s2")
                    nc.vector.tensor_add(s2[:], p1[:],
                                         hl_cur[0:2, 96 + Bc:96 + 2 * Bc])
                    nc.vector.tensor_add(xf[:], xf[:], s2[:])
                    nc.vector.tensor_copy(outb[:, Bc * (s - 1):Bc * s], xf[:])

                if gru:
                    h = h_new
                ra_cur, za_cur, nb_cur = ra_next, za_next, nb_next
                hl_cur = hl_next

            nc.sync.dma_start(out=d_out[:], in_=outb[:])

    nc.compile()
    return nc


# ------------------------------------------------------------------ interface

def kernel(X0, V, W_ih, W_hh, b_ih, b_hh, Wa1, ba1, Wa2, ba2, Wr, br,
           _trace=False, _tmpdir=None):
    from concourse.bass_utils import run_bass_kernel_spmd

    if "prog" not in _PROG_CACHE:
        _PROG_CACHE["prog"] = _build_program()
    nc = _PROG_CACHE["prog"]

    consts = _prep_consts(W_ih, W_hh, b_ih, b_hh, Wa1, ba1, Wa2, ba2, Wr, br)
    in_maps = []
    for c in range(NCORES):
        core = _prep_core(c, X0, V)
        in_maps.append({**consts, **core})

    res = run_bass_kernel_spmd(nc, in_maps, list(range(NCORES)),
                               trace=_trace, tmpdir=_tmpdir)
    outs = []
    for c in range(NCORES):
        buf = np.asarray(res.results[c]["out"], dtype=np.float32)  # [2, T*Bc]
        outs.append(buf.reshape(2, T, Bc).transpose(2, 1, 0))
    out = np.concatenate(outs, axis=0)
    if _trace:
        return out, res
    return out


# revision 5
# speedup vs baseline: 1.0860x; 1.0013x over previous
"""AttentionResidualGRU fused Trainium2 kernel — v9: latency-ordered.

Data parallel over batch (8 cores x 32 rows), feature-major state
(partition = feature, free = batch), weight-stationary recurrent matmuls
(48 LDW+MM pairs/step at the ~27ns N<=64 dispatch floor), scheduled with
the ASAP tile scheduler (dep-driven semaphores, not sim placement).

Per-step schedule around the loop-carried h chain:
  - PE pair order r(16), n(16), z(16): sigmoid(r) fires after only 16
    pairs, and the B bank (b_hn + gh_n) lands exactly when u = r*B + gN
    needs it; r-pairs are k-grouped in halves so the first 8 need only
    h chunks 0-1, which the split gate tail produces early.
  - rA, zA, [gN|B], hl each own a full PSUM bank (bufs=2): PSUM dep
    tracking is tile-granular, so sharing would chain sigmoid(r) behind
    the z pairs.
  - sigma(-z) yields oz = 1-z directly; post-tanh tail is t2 = oz*n ;
    h' = t2 - nq (column halves), with nq = (oz-1)*h = -z*h built
    off-chain by one fused scalar_tensor_tensor while tanh runs. u is
    written to nb-bank scratch columns so tanh reads PSUM (faster Act
    access than SBUF).
  - relu runs on the DVE (tensor_scalar_max) in its idle window so the
    logits matmul never blocks the PE stream behind the Act FIFO;
    scan2's elementwise tail runs on the DVE after h' reading the E
    rows straight from PSUM (GpSimd's per-op chain latency made it the
    critical x-loop).
  - gx projections for step s+1 and the attention v-part ride in the
    PE idle window at the end of step s (PSUM bufs=2 rotation).
"""

import os
import sys

os.environ.setdefault("TILE_SCHEDULER", "asap")

import numpy as np
import ml_dtypes

BF16 = ml_dtypes.bfloat16

for _p in ("/opt/trn_rl_repo", "/root/.axon_site/_ro/trn_rl_repo"):
    if os.path.isdir(_p) and _p not in sys.path:
        sys.path.append(_p)

B, T, H, IN, OUT = 256, 1024, 512, 2, 2
NCORES = 8
Bc = B // NCORES          # 32
SV_COLS = Bc * T          # 32768

_PROG_CACHE = {}


# ----------------------------------------------------------------- host prep

def _prep_consts(W_ih, W_hh, b_ih, b_hh, Wa1, ba1, Wa2, ba2, Wr, br):
    f = np.float32
    W_ih = np.asarray(W_ih, f); W_hh = np.asarray(W_hh, f)
    b_ih = np.asarray(b_ih, f); b_hh = np.asarray(b_hh, f)
    Wa1 = np.asarray(Wa1, f); ba1 = np.asarray(ba1, f)
    Wa2 = np.asarray(Wa2, f); ba2 = np.asarray(ba2, f)
    Wr = np.asarray(Wr, f); br = np.asarray(br, f)

    def gate_row0(m):
        return 128 * m if m < 4 else (512 + 128 * (m - 4) if m < 8
                                      else 1024 + 128 * (m - 8))

    # 48 stationary gate tiles: tile (m, k) at cols 128*(4m+k).
    wt = np.zeros((128, 48 * 128), f)
    for m in range(12):
        r0 = gate_row0(m)
        for k in range(4):
            wt[:, 128 * (4 * m + k):128 * (4 * m + k + 1)] = \
                W_hh[r0:r0 + 128, 128 * k:128 * (k + 1)].T

    b4 = np.zeros((4, 128), f)       # b_hh n-part, chunk k in row k
    for k in range(4):
        b4[k] = b_hh[1024 + 128 * k:1024 + 128 * (k + 1)]
    sel4 = np.zeros((4, 128), f)     # chunk selector rhs
    for c in range(4):
        sel4[c, 32 * c:32 * (c + 1)] = 1.0

    # in-loop gx stationaries: [3, 128] per chunk c (rows [bias, W0, W1])
    wx = np.zeros((3, 12 * 128), f)
    for c in range(12):
        r0 = gate_row0(c)
        bias = b_ih[r0:r0 + 128] + (b_hh[r0:r0 + 128] if c < 8 else 0.0)
        wx[0, 128 * c:128 * (c + 1)] = bias
        wx[1, 128 * c:128 * (c + 1)] = W_ih[r0:r0 + 128, 0]
        wx[2, 128 * c:128 * (c + 1)] = W_ih[r0:r0 + 128, 1]

    # E matmuls: E2 = [res0, res1 | v0, v1]
    ew = np.zeros((128, 8), f)       # k-tile k at cols 2k: [Wr0, Wr1]
    for k in range(4):
        ew[:, 2 * k + 0] = Wr[0, 128 * k:128 * (k + 1)]
        ew[:, 2 * k + 1] = Wr[1, 128 * k:128 * (k + 1)]
    eaP = np.zeros((3, 2), f)        # over [1, v0, v1]: br row
    eaP[0] = [br[0], br[1]]
    eaQ = np.zeros((3, 2), f)        # [v0, v1]
    eaQ[1, 0] = 1.0
    eaQ[2, 1] = 1.0

    # attention MLP split: v-part [3, 128]x2 (rows [ba1, Wa1v0, Wa1v1]),
    # x-part [2, 128]x2 (rows [Wa1x0, Wa1x1])
    wa1v = np.zeros((3, 256), f)
    wa1x = np.zeros((2, 256), f)
    for j in range(2):
        sl = slice(128 * j, 128 * (j + 1))
        wa1v[0, sl] = ba1[sl]
        wa1v[1, sl] = Wa1[sl, 2]
        wa1v[2, sl] = Wa1[sl, 3]
        wa1x[0, sl] = Wa1[sl, 0]
        wa1x[1, sl] = Wa1[sl, 1]
    wd = (Wa2[0] - Wa2[1]).astype(f)  # [256]
    wd2 = np.zeros((128, 4), f)       # k-tile k at cols 2k: [wd, -wd]
    for k in range(2):
        wd2[:, 2 * k + 0] = wd[128 * k:128 * (k + 1)]
        wd2[:, 2 * k + 1] = -wd[128 * k:128 * (k + 1)]
    db = float(ba2[0] - ba2[1])
    dbias = np.array([[db], [-db]], f)

    return dict(wt=wt.astype(BF16),
                b4=b4.astype(BF16), sel4=sel4.astype(BF16),
                wx=wx.astype(BF16), ew=ew.astype(BF16),
                eaP=eaP.astype(BF16), eaQ=eaQ.astype(BF16),
                wa1v=wa1v.astype(BF16), wa1x=wa1x.astype(BF16),
                wd2=wd2.astype(BF16), dbias=dbias)


def _prep_core(c, X0, V):
    f = np.float32
    bs = slice(Bc * c, Bc * (c + 1))
    Vc = np.asarray(V[bs], f)                      # [32, T, 2]
    sv3 = np.zeros((3, SV_COLS), f)                # rows [1, v0, v1], col 32t+b
    sv3[0] = 1.0
    sv3[1] = Vc[:, :, 0].T.reshape(-1)
    sv3[2] = Vc[:, :, 1].T.reshape(-1)
    xf0 = np.asarray(X0[bs], f).T.copy()           # [2, 32] f32
    return dict(sv3=sv3.astype(BF16), xf0=xf0, xb0=xf0.astype(BF16))


# ------------------------------------------------------------- device program

def _build_program():
    from concourse import bacc, tile, mybir  # noqa

    f32 = mybir.dt.float32
    bf16 = mybir.dt.bfloat16
    AF = mybir.ActivationFunctionType
    ALU = mybir.AluOpType

    nc = bacc.Bacc(None)
    d_wt = nc.declare_dram_parameter("wt", [128, 48 * 128], bf16, isOutput=False)
    d_b4 = nc.declare_dram_parameter("b4", [4, 128], bf16, isOutput=False)
    d_sel = nc.declare_dram_parameter("sel4", [4, 128], bf16, isOutput=False)
    d_wx = nc.declare_dram_parameter("wx", [3, 12 * 128], bf16, isOutput=False)
    d_ew = nc.declare_dram_parameter("ew", [128, 8], bf16, isOutput=False)
    d_eaP = nc.declare_dram_parameter("eaP", [3, 2], bf16, isOutput=False)
    d_eaQ = nc.declare_dram_parameter("eaQ", [3, 2], bf16, isOutput=False)
    d_wa1v = nc.declare_dram_parameter("wa1v", [3, 256], bf16, isOutput=False)
    d_wa1x = nc.declare_dram_parameter("wa1x", [2, 256], bf16, isOutput=False)
    d_wd2 = nc.declare_dram_parameter("wd2", [128, 4], bf16, isOutput=False)
    d_db = nc.declare_dram_parameter("dbias", [2, 1], f32, isOutput=False)
    d_sv = nc.declare_dram_parameter("sv3", [3, SV_COLS], bf16, isOutput=False)
    d_xf0 = nc.declare_dram_parameter("xf0", [2, Bc], f32, isOutput=False)
    d_xb0 = nc.declare_dram_parameter("xb0", [2, Bc], bf16, isOutput=False)
    d_out = nc.declare_dram_parameter("out", [2, T * Bc], bf16, isOutput=True)

    with tile.TileContext(nc) as tc:
        with (
            tc.tile_pool(name="const", bufs=1) as cpool,
            tc.tile_pool(name="state", bufs=1) as spool,
            tc.tile_pool(name="hpool", bufs=2) as hpool,
            tc.tile_pool(name="work", bufs=2) as wpool,
            tc.tile_pool(name="pRA", bufs=2, space="PSUM") as pRA,
            tc.tile_pool(name="pZA", bufs=2, space="PSUM") as pZA,
            tc.tile_pool(name="pNB", bufs=2, space="PSUM") as pNB,
            tc.tile_pool(name="pHL", bufs=2, space="PSUM") as pHL,
        ):
            # ---- constants
            wt = cpool.tile([128, 48 * 128], bf16, tag="wt")
            b4_t = cpool.tile([4, 128], bf16, tag="b4")
            sel_t = cpool.tile([4, 128], bf16, tag="sel4")
            wx_t = cpool.tile([3, 12 * 128], bf16, tag="wx")
            ew_t = cpool.tile([128, 8], bf16, tag="ew")
            eaP_t = cpool.tile([3, 2], bf16, tag="eaP")
            eaQ_t = cpool.tile([3, 2], bf16, tag="eaQ")
            wa1v_t = cpool.tile([3, 256], bf16, tag="wa1v")
            wa1x_t = cpool.tile([2, 256], bf16, tag="wa1x")
            wd2_t = cpool.tile([128, 4], bf16, tag="wd2")
            db_t = cpool.tile([2, 1], f32, tag="dbias")
            sv_t = cpool.tile([3, SV_COLS], bf16, tag="sv3")
            for dst, src in ((b4_t, d_b4),
                             (sel_t, d_sel), (wx_t, d_wx), (ew_t, d_ew),
                             (eaP_t, d_eaP), (eaQ_t, d_eaQ), (wa1v_t, d_wa1v),
                             (wa1x_t, d_wa1x), (wd2_t, d_wd2), (db_t, d_db),
                             (sv_t, d_sv)):
                nc.sync.dma_start(out=dst[:], in_=src[:])
            # wt split into 4 column-range DMAs so the first r/n pairs
            # start before the full 1.5MB weight load completes
            for q in range(4):
                cs = slice(1536 * q, 1536 * (q + 1))
                nc.sync.dma_start(out=wt[:, cs], in_=d_wt[:, cs])

            # ---- state
            xf = spool.tile([2, Bc], f32, tag="xf")
            outb = spool.tile([2, T * Bc], bf16, tag="outb")
            xb0 = spool.tile([2, Bc], bf16, tag="xb0")
            nc.sync.dma_start(out=xf[:], in_=d_xf0[:])
            nc.sync.dma_start(out=xb0[:], in_=d_xb0[:])

            mm = nc.tensor.matmul

            h = hpool.tile([128, 128], bf16, tag="h")
            nc.vector.memset(h[:], 0.0)

            # gate m-chunk layout in the shared PSUM banks (tiles padded to a
            # full 2KB bank so double-buffered tiles never share a bank —
            # start=True clears has_written bank-wide):
            #   rz[:, 0:128]   = rA (m 0..3)     rz[:, 128:256] = zA (m 4..7)
            #   nb[:, 0:128]   = gN (m 8..11)    nb[:, 128:256] = B  (gh_n+b_hn)
            def gx_emit(ra, za, nb, col):
                svs = sv_t[:, Bc * col:Bc * (col + 1)]
                for m in range(12):
                    if m < 4:
                        dst = ra[:, 32 * m:32 * (m + 1)]
                    elif m < 8:
                        dst = za[:, 32 * (m - 4):32 * (m - 3)]
                    else:
                        dst = nb[:, 32 * (m - 8):32 * (m - 7)]
                    mm(dst, wx_t[:, 128 * m:128 * (m + 1)], svs,
                       start=(m in (0, 4, 8)), stop=False)
                mm(nb[:, 128:256], b4_t[:], sel_t[:], start=False, stop=False)

            # preamble: gx for step 0
            ra_cur = pRA.tile([128, 512], f32, tag="ra")
            za_cur = pZA.tile([128, 512], f32, tag="za")
            nb_cur = pNB.tile([128, 512], f32, tag="nb")
            gx_emit(ra_cur, za_cur, nb_cur, 0)

            hl_cur = None
            for s in range(T + 1):
                gru = s < T

                # ---- PE: r(16), n(16), z(16) pairs ----
                if gru:
                    # r-pairs split by h-chunk halves: the first 8 read only
                    # h chunks 0-1 so they can start on the early h' half
                    for ks in ((0, 1), (2, 3)):
                        for m in (0, 1, 2, 3):
                            dst = ra_cur[:, 32 * m:32 * (m + 1)]
                            for k in ks:
                                mm(dst,
                                   wt[:, 128 * (4 * m + k):128 * (4 * m + k + 1)],
                                   h[:, 32 * k:32 * (k + 1)],
                                   start=False, stop=(k == 3))
                    for m in (8, 9, 10, 11, 4, 5, 6, 7):
                        if m >= 8:
                            dst = nb_cur[:, 128 + 32 * (m - 8):128 + 32 * (m - 7)]
                        else:
                            dst = za_cur[:, 32 * (m - 4):32 * (m - 3)]
                        for k in range(4):
                            mm(dst,
                               wt[:, 128 * (4 * m + k):128 * (4 * m + k + 1)],
                               h[:, 32 * k:32 * (k + 1)],
                               start=False, stop=(k == 3))

                # ---- PE: attention x-part, E rows (scan2 output s-1) ----
                if s >= 1:
                    svp = sv_t[:, Bc * (s - 1):Bc * s]
                    # x-part accumulates onto last iteration's v-part; must
                    # precede E's start=True (bank-wide has_written clear)
                    xprev = (xb0[:] if s == 1
                             else outb[:, Bc * (s - 2):Bc * (s - 1)])
                    for j in range(2):
                        mm(hl_cur[:, 32 * j:32 * (j + 1)],
                           wa1x_t[:, 128 * j:128 * (j + 1)], xprev,
                           start=False, stop=True)
                    # E rows from h_prev (= h before this step's update)
                    for k in range(4):
                        mm(hl_cur[0:2, 96:96 + Bc], ew_t[:, 2 * k:2 * (k + 1)],
                           h[:, 32 * k:32 * (k + 1)],
                           start=(k == 0), stop=False)
                    mm(hl_cur[0:2, 96:96 + Bc], eaP_t[:], svp,
                       start=False, stop=True)
                    mm(hl_cur[0:2, 96 + Bc:96 + 2 * Bc], eaQ_t[:], svp,
                       start=True, stop=True)

                # ---- PE: gx + B init for step s+1 ----
                if s + 1 < T:
                    ra_next = pRA.tile([128, 512], f32, tag="ra")
                    za_next = pZA.tile([128, 512], f32, tag="za")
                    nb_next = pNB.tile([128, 512], f32, tag="nb")
                    gx_emit(ra_next, za_next, nb_next, s + 1)
                else:
                    ra_next = za_next = nb_next = None

                # ---- PE: attention v-part for iteration s+1 (svp' = v_s) ----
                if s < T:
                    hl_next = pHL.tile([128, 512], f32, tag="HL")
                    svn = sv_t[:, Bc * s:Bc * (s + 1)]
                    for j in range(2):
                        mm(hl_next[:, 32 * j:32 * (j + 1)],
                           wa1v_t[:, 128 * j:128 * (j + 1)], svn,
                           start=True, stop=False)
                else:
                    hl_next = None

                # ---- gate head: sigmoids, u = r*B + gN ----
                if gru:
                    r_sb = wpool.tile([128, 128], bf16, tag="r_sb")
                    nc.scalar.activation(r_sb[:], ra_cur[:, 0:128], AF.Sigmoid)
                    oz = wpool.tile([128, 128], bf16, tag="oz")
                    nc.scalar.activation(oz[:], za_cur[:, 0:128],
                                         AF.Sigmoid, scale=-1.0)
                    t1 = wpool.tile([128, 128], f32, tag="t1")
                    nc.vector.tensor_mul(t1[:], r_sb[:], nb_cur[:, 128:256])
                    # u goes to the nb bank's scratch columns: tanh reads
                    # PSUM with ~80 fewer access cycles than SBUF
                    u = nb_cur[:, 256:384]
                    nc.vector.tensor_add(u, t1[:], nb_cur[:, 0:128])

                # relu on DVE (fills its idle window; keeps Act FIFO short
                # and lets wd2 clear the PE stream early)
                if s >= 1:
                    hid = wpool.tile([128, 64], bf16, tag="hid")
                    nc.vector.tensor_scalar_max(hid[:], hl_cur[:, 0:64], 0.0)
                    for k in range(2):
                        mm(hl_cur[0:2, 64:96], wd2_t[:, 2 * k:2 * (k + 1)],
                           hid[:, 32 * k:32 * (k + 1)],
                           start=(k == 0), stop=(k == 1))

                if gru:
                    n_sb = wpool.tile([128, 128], bf16, tag="n_sb")
                    nc.scalar.activation(n_sb[:], u, AF.Tanh)
                    # off-chain while tanh runs: nq = (oz-1)*h = -z*h
                    nq = wpool.tile([128, 128], bf16, tag="nq")
                    nc.vector.scalar_tensor_tensor(
                        nq[:], oz[:], 1.0, h[:], ALU.subtract, ALU.mult)
                if s >= 1:
                    aw = wpool.tile([2, Bc], f32, tag="aw")
                    nc.scalar.activation(aw[:], hl_cur[0:2, 64:96], AF.Sigmoid,
                                         bias=db_t[:])

                # ---- gate tail: h' = oz*n - nq, column halves so the
                # first half of h' unblocks next iteration's k0/k1 pairs ----
                if gru:
                    t2 = wpool.tile([128, 128], bf16, tag="t2")
                    h_new = hpool.tile([128, 128], bf16, tag="h")
                    nc.vector.tensor_mul(t2[:, 0:64], oz[:, 0:64], n_sb[:, 0:64])
                    nc.vector.tensor_sub(h_new[:, 0:64], t2[:, 0:64],
                                         nq[:, 0:64])
                    nc.vector.tensor_mul(t2[:, 64:128], oz[:, 64:128],
                                         n_sb[:, 64:128])
                    nc.vector.tensor_sub(h_new[:, 64:128], t2[:, 64:128],
                                         nq[:, 64:128])

                # ---- scan2 elementwise tail (DVE idle window after h') ----
                if s >= 1:
                    p1 = wpool.tile([2, Bc], f32, tag="p1")
                    nc.vector.tensor_mul(p1[:], aw[:], hl_cur[0:2, 96:96 + Bc])
                    s2 = wpool.tile([2, Bc], f32, tag="s2")
                    nc.vector.tensor_add(s2[:], p1[:],
                                         hl_cur[0:2, 96 + Bc:96 + 2 * Bc])
                    nc.vector.tensor_add(xf[:], xf[:], s2[:])
                    nc.vector.tensor_copy(outb[:, Bc * (s - 1):Bc * s], xf[:])

                if gru:
                    h = h_new
                ra_cur, za_cur, nb_cur = ra_next, za_next, nb_next
                hl_cur = hl_next

            nc.sync.dma_start(out=d_out[:], in_=outb[:])

    nc.compile()
    return nc


# ------------------------------------------------------------------ interface

def kernel(X0, V, W_ih, W_hh, b_ih, b_hh, Wa1, ba1, Wa2, ba2, Wr, br,
           _trace=False, _tmpdir=None):
    from concourse.bass_utils import run_bass_kernel_spmd

    if "prog" not in _PROG_CACHE:
        _PROG_CACHE["prog"] = _build_program()
    nc = _PROG_CACHE["prog"]

    consts = _prep_consts(W_ih, W_hh, b_ih, b_hh, Wa1, ba1, Wa2, ba2, Wr, br)
    in_maps = []
    for c in range(NCORES):
        core = _prep_core(c, X0, V)
        in_maps.append({**consts, **core})

    res = run_bass_kernel_spmd(nc, in_maps, list(range(NCORES)),
                               trace=_trace, tmpdir=_tmpdir)
    outs = []
    for c in range(NCORES):
        buf = np.asarray(res.results[c]["out"], dtype=np.float32)  # [2, T*Bc]
        outs.append(buf.reshape(2, T, Bc).transpose(2, 1, 0))
    out = np.concatenate(outs, axis=0)
    if _trace:
        return out, res
    return out


# revision 6
# speedup vs baseline: 1.0861x; 1.0001x over previous
"""AttentionResidualGRU fused Trainium2 kernel — v9: latency-ordered.

Data parallel over batch (8 cores x 32 rows), feature-major state
(partition = feature, free = batch), weight-stationary recurrent matmuls
(48 LDW+MM pairs/step at the ~27ns N<=64 dispatch floor), scheduled with
the ASAP tile scheduler (dep-driven semaphores, not sim placement).

Per-step schedule around the loop-carried h chain:
  - PE pair order r(16), n(16), z(16): sigmoid(r) fires after only 16
    pairs, and the B bank (b_hn + gh_n) lands exactly when u = r*B + gN
    needs it; r-pairs are k-grouped in halves so the first 8 need only
    h chunks 0-1, which the split gate tail produces early.
  - rA, zA, [gN|B], hl each own a full PSUM bank (bufs=2): PSUM dep
    tracking is tile-granular, so sharing would chain sigmoid(r) behind
    the z pairs.
  - sigma(-z) yields oz = 1-z directly; post-tanh tail is t2 = oz*n ;
    h' = t2 - nq (column halves), with nq = (oz-1)*h = -z*h built
    off-chain by one fused scalar_tensor_tensor while tanh runs. u is
    written to nb-bank scratch columns so tanh reads PSUM (faster Act
    access than SBUF).
  - relu runs on the DVE (tensor_scalar_max) in its idle window so the
    logits matmul never blocks the PE stream behind the Act FIFO;
    scan2's elementwise tail runs on the DVE after h' reading the E
    rows straight from PSUM (GpSimd's per-op chain latency made it the
    critical x-loop).
  - gx projections for step s+1 and the attention v-part ride in the
    PE idle window at the end of step s (PSUM bufs=2 rotation).
"""

import os
import sys

os.environ.setdefault("TILE_SCHEDULER", "asap")

import numpy as np
import ml_dtypes

BF16 = ml_dtypes.bfloat16

for _p in ("/opt/trn_rl_repo", "/root/.axon_site/_ro/trn_rl_repo"):
    if os.path.isdir(_p) and _p not in sys.path:
        sys.path.append(_p)

B, T, H, IN, OUT = 256, 1024, 512, 2, 2
NCORES = 8
Bc = B // NCORES          # 32
SV_COLS = Bc * T          # 32768

_PROG_CACHE = {}


# ----------------------------------------------------------------- host prep

def _prep_consts(W_ih, W_hh, b_ih, b_hh, Wa1, ba1, Wa2, ba2, Wr, br):
    f = np.float32
    W_ih = np.asarray(W_ih, f); W_hh = np.asarray(W_hh, f)
    b_ih = np.asarray(b_ih, f); b_hh = np.asarray(b_hh, f)
    Wa1 = np.asarray(Wa1, f); ba1 = np.asarray(ba1, f)
    Wa2 = np.asarray(Wa2, f); ba2 = np.asarray(ba2, f)
    Wr = np.asarray(Wr, f); br = np.asarray(br, f)

    def gate_row0(m):
        return 128 * m if m < 4 else (512 + 128 * (m - 4) if m < 8
                                      else 1024 + 128 * (m - 8))

    # 48 stationary gate tiles: tile (m, k) at cols 128*(4m+k).
    wt = np.zeros((128, 48 * 128), f)
    for m in range(12):
        r0 = gate_row0(m)
        for k in range(4):
            wt[:, 128 * (4 * m + k):128 * (4 * m + k + 1)] = \
                W_hh[r0:r0 + 128, 128 * k:128 * (k + 1)].T

    b4 = np.zeros((4, 128), f)       # b_hh n-part, chunk k in row k
    for k in range(4):
        b4[k] = b_hh[1024 + 128 * k:1024 + 128 * (k + 1)]
    sel4 = np.zeros((4, 128), f)     # chunk selector rhs
    for c in range(4):
        sel4[c, 32 * c:32 * (c + 1)] = 1.0

    # in-loop gx stationaries: [3, 128] per chunk c (rows [bias, W0, W1])
    wx = np.zeros((3, 12 * 128), f)
    for c in range(12):
        r0 = gate_row0(c)
        bias = b_ih[r0:r0 + 128] + (b_hh[r0:r0 + 128] if c < 8 else 0.0)
        wx[0, 128 * c:128 * (c + 1)] = bias
        wx[1, 128 * c:128 * (c + 1)] = W_ih[r0:r0 + 128, 0]
        wx[2, 128 * c:128 * (c + 1)] = W_ih[r0:r0 + 128, 1]

    # E matmuls: E2 = [res0, res1 | v0, v1]
    ew = np.zeros((128, 8), f)       # k-tile k at cols 2k: [Wr0, Wr1]
    for k in range(4):
        ew[:, 2 * k + 0] = Wr[0, 128 * k:128 * (k + 1)]
        ew[:, 2 * k + 1] = Wr[1, 128 * k:128 * (k + 1)]
    eaP = np.zeros((3, 2), f)        # over [1, v0, v1]: br row
    eaP[0] = [br[0], br[1]]
    eaQ = np.zeros((3, 2), f)        # [v0, v1]
    eaQ[1, 0] = 1.0
    eaQ[2, 1] = 1.0

    # attention MLP split: v-part [3, 128]x2 (rows [ba1, Wa1v0, Wa1v1]),
    # x-part [2, 128]x2 (rows [Wa1x0, Wa1x1])
    wa1v = np.zeros((3, 256), f)
    wa1x = np.zeros((2, 256), f)
    for j in range(2):
        sl = slice(128 * j, 128 * (j + 1))
        wa1v[0, sl] = ba1[sl]
        wa1v[1, sl] = Wa1[sl, 2]
        wa1v[2, sl] = Wa1[sl, 3]
        wa1x[0, sl] = Wa1[sl, 0]
        wa1x[1, sl] = Wa1[sl, 1]
    wd = (Wa2[0] - Wa2[1]).astype(f)  # [256]
    wd2 = np.zeros((128, 4), f)       # k-tile k at cols 2k: [wd, -wd]
    for k in range(2):
        wd2[:, 2 * k + 0] = wd[128 * k:128 * (k + 1)]
        wd2[:, 2 * k + 1] = -wd[128 * k:128 * (k + 1)]
    db = float(ba2[0] - ba2[1])
    dbias = np.array([[db], [-db]], f)

    return dict(wt=wt.astype(BF16),
                b4=b4.astype(BF16), sel4=sel4.astype(BF16),
                wx=wx.astype(BF16), ew=ew.astype(BF16),
                eaP=eaP.astype(BF16), eaQ=eaQ.astype(BF16),
                wa1v=wa1v.astype(BF16), wa1x=wa1x.astype(BF16),
                wd2=wd2.astype(BF16), dbias=dbias)


def _prep_core(c, X0, V):
    f = np.float32
    bs = slice(Bc * c, Bc * (c + 1))
    Vc = np.asarray(V[bs], f)                      # [32, T, 2]
    sv3 = np.zeros((3, SV_COLS), f)                # rows [1, v0, v1], col 32t+b
    sv3[0] = 1.0
    sv3[1] = Vc[:, :, 0].T.reshape(-1)
    sv3[2] = Vc[:, :, 1].T.reshape(-1)
    xf0 = np.asarray(X0[bs], f).T.copy()           # [2, 32] f32
    return dict(sv3=sv3.astype(BF16), xf0=xf0, xb0=xf0.astype(BF16))


# ------------------------------------------------------------- device program

def _build_program():
    from concourse import bacc, tile, mybir  # noqa

    f32 = mybir.dt.float32
    bf16 = mybir.dt.bfloat16
    AF = mybir.ActivationFunctionType
    ALU = mybir.AluOpType

    nc = bacc.Bacc(None)
    d_wt = nc.declare_dram_parameter("wt", [128, 48 * 128], bf16, isOutput=False)
    d_b4 = nc.declare_dram_parameter("b4", [4, 128], bf16, isOutput=False)
    d_sel = nc.declare_dram_parameter("sel4", [4, 128], bf16, isOutput=False)
    d_wx = nc.declare_dram_parameter("wx", [3, 12 * 128], bf16, isOutput=False)
    d_ew = nc.declare_dram_parameter("ew", [128, 8], bf16, isOutput=False)
    d_eaP = nc.declare_dram_parameter("eaP", [3, 2], bf16, isOutput=False)
    d_eaQ = nc.declare_dram_parameter("eaQ", [3, 2], bf16, isOutput=False)
    d_wa1v = nc.declare_dram_parameter("wa1v", [3, 256], bf16, isOutput=False)
    d_wa1x = nc.declare_dram_parameter("wa1x", [2, 256], bf16, isOutput=False)
    d_wd2 = nc.declare_dram_parameter("wd2", [128, 4], bf16, isOutput=False)
    d_db = nc.declare_dram_parameter("dbias", [2, 1], f32, isOutput=False)
    d_sv = nc.declare_dram_parameter("sv3", [3, SV_COLS], bf16, isOutput=False)
    d_xf0 = nc.declare_dram_parameter("xf0", [2, Bc], f32, isOutput=False)
    d_xb0 = nc.declare_dram_parameter("xb0", [2, Bc], bf16, isOutput=False)
    d_out = nc.declare_dram_parameter("out", [2, T * Bc], bf16, isOutput=True)

    with tile.TileContext(nc) as tc:
        with (
            tc.tile_pool(name="const", bufs=1) as cpool,
            tc.tile_pool(name="state", bufs=1) as spool,
            tc.tile_pool(name="hpool", bufs=2) as hpool,
            tc.tile_pool(name="work", bufs=2) as wpool,
            tc.tile_pool(name="pRA", bufs=2, space="PSUM") as pRA,
            tc.tile_pool(name="pZA", bufs=2, space="PSUM") as pZA,
            tc.tile_pool(name="pNB", bufs=2, space="PSUM") as pNB,
            tc.tile_pool(name="pHL", bufs=2, space="PSUM") as pHL,
        ):
            # ---- constants
            wt = cpool.tile([128, 48 * 128], bf16, tag="wt")
            b4_t = cpool.tile([4, 128], bf16, tag="b4")
            sel_t = cpool.tile([4, 128], bf16, tag="sel4")
            wx_t = cpool.tile([3, 12 * 128], bf16, tag="wx")
            ew_t = cpool.tile([128, 8], bf16, tag="ew")
            eaP_t = cpool.tile([3, 2], bf16, tag="eaP")
            eaQ_t = cpool.tile([3, 2], bf16, tag="eaQ")
            wa1v_t = cpool.tile([3, 256], bf16, tag="wa1v")
            wa1x_t = cpool.tile([2, 256], bf16, tag="wa1x")
            wd2_t = cpool.tile([128, 4], bf16, tag="wd2")
            db_t = cpool.tile([2, 1], f32, tag="dbias")
            sv_t = cpool.tile([3, SV_COLS], bf16, tag="sv3")
            # DMA order: the preamble gx needs sv3+wx first; recurrent
            # pairs need b4/sel and the wt quarters; scan2 constants are
            # not consumed until iteration 1 so they go last
            for dst, src in ((sv_t, d_sv), (wx_t, d_wx),
                             (b4_t, d_b4), (sel_t, d_sel)):
                nc.sync.dma_start(out=dst[:], in_=src[:])
            for q in range(4):
                cs = slice(1536 * q, 1536 * (q + 1))
                nc.sync.dma_start(out=wt[:, cs], in_=d_wt[:, cs])
            for dst, src in ((ew_t, d_ew),
                             (eaP_t, d_eaP), (eaQ_t, d_eaQ), (wa1v_t, d_wa1v),
                             (wa1x_t, d_wa1x), (wd2_t, d_wd2), (db_t, d_db)):
                nc.sync.dma_start(out=dst[:], in_=src[:])

            # ---- state
            xf = spool.tile([2, Bc], f32, tag="xf")
            outb = spool.tile([2, T * Bc], bf16, tag="outb")
            xb0 = spool.tile([2, Bc], bf16, tag="xb0")
            nc.sync.dma_start(out=xf[:], in_=d_xf0[:])
            nc.sync.dma_start(out=xb0[:], in_=d_xb0[:])

            mm = nc.tensor.matmul

            h = hpool.tile([128, 128], bf16, tag="h")
            nc.vector.memset(h[:], 0.0)

            # gate m-chunk layout in the shared PSUM banks (tiles padded to a
            # full 2KB bank so double-buffered tiles never share a bank —
            # start=True clears has_written bank-wide):
            #   rz[:, 0:128]   = rA (m 0..3)     rz[:, 128:256] = zA (m 4..7)
            #   nb[:, 0:128]   = gN (m 8..11)    nb[:, 128:256] = B  (gh_n+b_hn)
            def gx_emit(ra, za, nb, col):
                svs = sv_t[:, Bc * col:Bc * (col + 1)]
                for m in range(12):
                    if m < 4:
                        dst = ra[:, 32 * m:32 * (m + 1)]
                    elif m < 8:
                        dst = za[:, 32 * (m - 4):32 * (m - 3)]
                    else:
                        dst = nb[:, 32 * (m - 8):32 * (m - 7)]
                    mm(dst, wx_t[:, 128 * m:128 * (m + 1)], svs,
                       start=(m in (0, 4, 8)), stop=False)
                mm(nb[:, 128:256], b4_t[:], sel_t[:], start=False, stop=False)

            # preamble: gx for step 0
            ra_cur = pRA.tile([128, 512], f32, tag="ra")
            za_cur = pZA.tile([128, 512], f32, tag="za")
            nb_cur = pNB.tile([128, 512], f32, tag="nb")
            gx_emit(ra_cur, za_cur, nb_cur, 0)

            hl_cur = None
            for s in range(T + 1):
                gru = s < T

                # ---- PE: r(16), n(16), z(16) pairs ----
                if gru:
                    # r-pairs split by h-chunk halves: the first 8 read only
                    # h chunks 0-1 so they can start on the early h' half
                    for ks in ((0, 1), (2, 3)):
                        for m in (0, 1, 2, 3):
                            dst = ra_cur[:, 32 * m:32 * (m + 1)]
                            for k in ks:
                                mm(dst,
                                   wt[:, 128 * (4 * m + k):128 * (4 * m + k + 1)],
                                   h[:, 32 * k:32 * (k + 1)],
                                   start=False, stop=(k == 3))
                    for m in (8, 9, 10, 11, 4, 5, 6, 7):
                        if m >= 8:
                            dst = nb_cur[:, 128 + 32 * (m - 8):128 + 32 * (m - 7)]
                        else:
                            dst = za_cur[:, 32 * (m - 4):32 * (m - 3)]
                        for k in range(4):
                            mm(dst,
                               wt[:, 128 * (4 * m + k):128 * (4 * m + k + 1)],
                               h[:, 32 * k:32 * (k + 1)],
                               start=False, stop=(k == 3))

                # ---- PE: attention x-part, E rows (scan2 output s-1) ----
                if s >= 1:
                    svp = sv_t[:, Bc * (s - 1):Bc * s]
                    # x-part accumulates onto last iteration's v-part; must
                    # precede E's start=True (bank-wide has_written clear)
                    xprev = (xb0[:] if s == 1
                             else outb[:, Bc * (s - 2):Bc * (s - 1)])
                    for j in range(2):
                        mm(hl_cur[:, 32 * j:32 * (j + 1)],
                           wa1x_t[:, 128 * j:128 * (j + 1)], xprev,
                           start=False, stop=True)
                    # E rows from h_prev (= h before this step's update)
                    for k in range(4):
                        mm(hl_cur[0:2, 96:96 + Bc], ew_t[:, 2 * k:2 * (k + 1)],
                           h[:, 32 * k:32 * (k + 1)],
                           start=(k == 0), stop=False)
                    mm(hl_cur[0:2, 96:96 + Bc], eaP_t[:], svp,
                       start=False, stop=True)
                    mm(hl_cur[0:2, 96 + Bc:96 + 2 * Bc], eaQ_t[:], svp,
                       start=True, stop=True)

                # ---- PE: gx + B init for step s+1 ----
                if s + 1 < T:
                    ra_next = pRA.tile([128, 512], f32, tag="ra")
                    za_next = pZA.tile([128, 512], f32, tag="za")
                    nb_next = pNB.tile([128, 512], f32, tag="nb")
                    gx_emit(ra_next, za_next, nb_next, s + 1)
                else:
                    ra_next = za_next = nb_next = None

                # ---- PE: attention v-part for iteration s+1 (svp' = v_s) ----
                if s < T:
                    hl_next = pHL.tile([128, 512], f32, tag="HL")
                    svn = sv_t[:, Bc * s:Bc * (s + 1)]
                    for j in range(2):
                        mm(hl_next[:, 32 * j:32 * (j + 1)],
                           wa1v_t[:, 128 * j:128 * (j + 1)], svn,
                           start=True, stop=False)
                else:
                    hl_next = None

                # ---- gate head: sigmoids, u = r*B + gN ----
                if gru:
                    r_sb = wpool.tile([128, 128], bf16, tag="r_sb")
                    nc.scalar.activation(r_sb[:], ra_cur[:, 0:128], AF.Sigmoid)
                    oz = wpool.tile([128, 128], bf16, tag="oz")
                    nc.scalar.activation(oz[:], za_cur[:, 0:128],
                                         AF.Sigmoid, scale=-1.0)
                    t1 = wpool.tile([128, 128], f32, tag="t1")
                    nc.vector.tensor_mul(t1[:], r_sb[:], nb_cur[:, 128:256])
                    # u goes to the nb bank's scratch columns: tanh reads
                    # PSUM with ~80 fewer access cycles than SBUF
                    u = nb_cur[:, 256:384]
                    nc.vector.tensor_add(u, t1[:], nb_cur[:, 0:128])

                # relu on DVE (fills its idle window; keeps Act FIFO short
                # and lets wd2 clear the PE stream early)
                if s >= 1:
                    hid = wpool.tile([128, 64], bf16, tag="hid")
                    nc.vector.tensor_scalar_max(hid[:], hl_cur[:, 0:64], 0.0)
                    for k in range(2):
                        mm(hl_cur[0:2, 64:96], wd2_t[:, 2 * k:2 * (k + 1)],
                           hid[:, 32 * k:32 * (k + 1)],
                           start=(k == 0), stop=(k == 1))

                if gru:
                    n_sb = wpool.tile([128, 128], bf16, tag="n_sb")
                    nc.scalar.activation(n_sb[:], u, AF.Tanh)
                    # off-chain while tanh runs: nq = (oz-1)*h = -z*h
                    nq = wpool.tile([128, 128], bf16, tag="nq")
                    nc.vector.scalar_tensor_tensor(
                        nq[:], oz[:], 1.0, h[:], ALU.subtract, ALU.mult)
                if s >= 1:
                    aw = wpool.tile([2, Bc], f32, tag="aw")
                    nc.scalar.activation(aw[:], hl_cur[0:2, 64:96], AF.Sigmoid,
                                         bias=db_t[:])

                # ---- gate tail: h' = oz*n - nq, column halves so the
                # first half of h' unblocks next iteration's k0/k1 pairs ----
                if gru:
                    t2 = wpool.tile([128, 128], bf16, tag="t2")
                    h_new = hpool.tile([128, 128], bf16, tag="h")
                    nc.vector.tensor_mul(t2[:, 0:64], oz[:, 0:64], n_sb[:, 0:64])
                    nc.vector.tensor_sub(h_new[:, 0:64], t2[:, 0:64],
                                         nq[:, 0:64])
                    nc.vector.tensor_mul(t2[:, 64:128], oz[:, 64:128],
                                         n_sb[:, 64:128])
                    nc.vector.tensor_sub(h_new[:, 64:128], t2[:, 64:128],
                                         nq[:, 64:128])

                # ---- scan2 elementwise tail (DVE idle window after h') ----
                if s >= 1:
                    p1 = wpool.tile([2, Bc], f32, tag="p1")
                    nc.vector.tensor_mul(p1[:], aw[:], hl_cur[0:2, 96:96 + Bc])
                    s2 = wpool.tile([2, Bc], f32, tag="s2")
                    nc.vector.tensor_add(s2[:], p1[:],
                                         hl_cur[0:2, 96 + Bc:96 + 2 * Bc])
                    nc.vector.tensor_add(xf[:], xf[:], s2[:])
                    nc.vector.tensor_copy(outb[:, Bc * (s - 1):Bc * s], xf[:])

                if gru:
                    h = h_new
                ra_cur, za_cur, nb_cur = ra_next, za_next, nb_next
                hl_cur = hl_next

            nc.sync.dma_start(out=d_out[:], in_=outb[:])

    nc.compile()
    return nc


# ------------------------------------------------------------------ interface

def kernel(X0, V, W_ih, W_hh, b_ih, b_hh, Wa1, ba1, Wa2, ba2, Wr, br,
           _trace=False, _tmpdir=None):
    from concourse.bass_utils import run_bass_kernel_spmd

    if "prog" not in _PROG_CACHE:
        _PROG_CACHE["prog"] = _build_program()
    nc = _PROG_CACHE["prog"]

    consts = _prep_consts(W_ih, W_hh, b_ih, b_hh, Wa1, ba1, Wa2, ba2, Wr, br)
    in_maps = []
    for c in range(NCORES):
        core = _prep_core(c, X0, V)
        in_maps.append({**consts, **core})

    res = run_bass_kernel_spmd(nc, in_maps, list(range(NCORES)),
                               trace=_trace, tmpdir=_tmpdir)
    outs = []
    for c in range(NCORES):
        buf = np.asarray(res.results[c]["out"], dtype=np.float32)  # [2, T*Bc]
        outs.append(buf.reshape(2, T, Bc).transpose(2, 1, 0))
    out = np.concatenate(outs, axis=0)
    if _trace:
        return out, res
    return out
